# revision 35
# baseline (speedup 1.0000x reference)
"""Deformable-conv Trainium2 kernel v3 (nn_DeformConv_11553462026367).

Data-parallel over batch: one sample per NeuronCore (8 cores).

v3 design (vs v2): host-staged gather table + software-pipelined prologue.
  - The 2x2 bilinear patch table (a pure relayout of x: 256 bf16 per
    record, elem = ch*4 + q, record (c, r) at DRAM row c*132 + r holding
    [x[r-2,c-2], x[r-2,c-1], x[r-1,c-2], x[r-1,c-1]], zero borders) is
    built on the HOST in prep_core_inputs and uploaded as an
    ExternalInput, removing the on-device table build (~25us of DMA
    writes + PE/Pool/Act prologue work).
  - The remaining prologue (offsets conv -> index math -> index wrap ->
    quadrant weights -> rdram staging) is emitted per QUARTER (32 s-cols
    = 2 chunks) and interleaved with the main gather/blend/conv loop, so
    the serialized-DMA gather stream starts ~20us in instead of ~110us.
  - Main loop unchanged in structure: per (chunk, k): transpose-mode
    dma_gather of 2048x512B records, PE expansion of quadrant weights,
    bf16 DVE blend, accumulating bf16 output matmuls in PSUM.

kernel(**inputs) takes the FULL batch and returns the FULL output.
"""
import sys
sys.path.insert(0, "/opt/trn_rl_repo")

import numpy as np
import ml_dtypes
from contextlib import ExitStack

from concourse import bass, tile
import concourse.bacc as bacc
from concourse.tile import add_dep_helper
import concourse.bass_utils as bass_utils
import concourse.mybir as mybir
from concourse.masks import make_identity

F32 = mybir.dt.float32
F32R = mybir.dt.float32r
BF16 = mybir.dt.bfloat16
F16 = mybir.dt.float16
I32 = mybir.dt.int32
I16 = mybir.dt.int16
ALU = mybir.AluOpType

# ---- problem constants (hardcoded; kernel.py must be self-contained) ----
B, C, H, W = 8, 64, 128, 128
KK = 9
HW = H * W                 # 16384 positions
LRr = 132                  # records per table column: r = clamp(y0+2) in [0,131]
LCc = 133                  # table columns: c = clamp(x0+2) in [0,132]
NREC = LCc * LRr           # 17556 records
NRECP = NREC + 8           # pad
CAST_RNE = True            # HW f32->i32 tensor_copy rounds-to-nearest (sim truncates)
NCORES = 8

NI = 2048                  # gather indices per call
NCHUNK = HW // NI          # 8
SS = NI // 128             # 16 s-slots (of 128 positions) per chunk
NQ = 4                     # quarters (32 s-cols each = 2 chunks)


def build_kernel(tc, outs, ins):
    nc = tc.nc
    ctx = ExitStack()
    with ctx:
        # ---------------- constants ----------------
        const_pool = ctx.enter_context(tc.tile_pool(name="const", bufs=1))
        ident = const_pool.tile([128, 128], F32)
        make_identity(nc, ident[:])
        identb = const_pool.tile([128, 128], BF16)
        nc.scalar.copy(identb[:], ident[:])

        piota_i = const_pool.tile([128, 1], I32)
        nc.gpsimd.iota(piota_i[:], pattern=[[0, 1]], base=0, channel_multiplier=1)
        piota = const_pool.tile([128, 1], F32)
        nc.vector.tensor_copy(piota[:], piota_i[:])
        siota_i = const_pool.tile([128, 128], I32)
        nc.gpsimd.iota(siota_i[:], pattern=[[1, 128]], base=0, channel_multiplier=0)
        siota = const_pool.tile([128, 128], F32)
        nc.vector.tensor_copy(siota[:], siota_i[:])

        # msel[pp][q, P] = 1 iff q == pp*16 + P%16  (wrap-permute one-hots)
        msel_f = const_pool.tile([128, 8 * 128], F32)
        msel_v = msel_f[:].rearrange("p (a b) -> p a b", a=8)
        clo16_i = const_pool.tile([128, 128], I32)
        nc.vector.tensor_scalar(clo16_i[:], siota_i[:], 15, None, ALU.bitwise_and)
        clo16 = const_pool.tile([128, 128], F32)
        nc.vector.tensor_copy(clo16[:], clo16_i[:])
        psh = const_pool.tile([128, 8], F32)
        for pp in range(8):
            nc.vector.tensor_scalar(psh[:, pp:pp + 1], piota[:], float(pp * 16),
                                    None, ALU.subtract)
            nc.vector.tensor_scalar(msel_v[:, pp, :], clo16[:], psh[:, pp:pp + 1],
                                    None, ALU.is_equal)

        # SEL36_k[(q*9+j), p] = 1 iff j == k and p%4 == q: selects k's four
        # quadrant rows out of the [36, *] chunk-weight tile.
        sel36_f = const_pool.tile([36, 9 * 128], F32)
        s36v = sel36_f[:].rearrange("p (k c) -> p k c", k=KK)
        clo4_36i = const_pool.tile([36, 128], I32)
        nc.vector.tensor_scalar(clo4_36i[:], siota_i[:36, :], 3, None,
                                ALU.bitwise_and)
        clo4_36 = const_pool.tile([36, 128], F32)
        nc.vector.tensor_copy(clo4_36[:], clo4_36i[:])
        qk_f = const_pool.tile([36, 1], F32)
        nc.vector.tensor_copy(qk_f[:], piota_i[:36, :])
        for k in range(KK):
            # partition q*9+j selects columns p with j == k, p%4 == q
            t36 = const_pool.tile([36, 128], F32)
            nc.vector.tensor_scalar(t36[:], clo4_36[:], 9.0, float(k),
                                    ALU.mult, ALU.add)
            nc.vector.tensor_scalar(s36v[:, k, :], t36[:], qk_f[:], None,
                                    ALU.is_equal)
        sel36 = const_pool.tile([36, 9 * 128], BF16)
        nc.scalar.copy(sel36[:], sel36_f[:])
        sel36_v = sel36[:].rearrange("p (k c) -> p k c", k=KK)

        # weights from host (offset-conv taps paired two-per-matmul)
        woff_f = const_pool.tile([128, 6 * 18], F32)
        nc.sync.dma_start(
            woff_f[:].rearrange("p (k o) -> p k o", k=6),
            ins["wpair"].transpose([1, 0, 2]))
        woff_sb = const_pool.tile([128, 6 * 18], F16)
        nc.scalar.copy(woff_sb[:], woff_f[:])
        woff_v = woff_sb[:].rearrange("p (k o) -> p k o", k=6)

        wdrep = const_pool.tile([128, 18 * 64], BF16)
        nc.sync.dma_start(
            wdrep[:].rearrange("p (i o) -> p i o", i=18),
            ins["wdrep"].transpose([1, 0, 2]))
        wdrep_v = wdrep[:].rearrange("p (i o) -> p i o", i=18)

        # ---------------- persistent tiles ----------------
        T_pool = ctx.enter_context(tc.tile_pool(name="persist", bufs=1))
        Ttile = T_pool.tile([128, 128 * 18], F32)          # offsets [p=w, s=h, ch]
        T3 = Ttile[:].rearrange("p (s c) -> p s c", c=18)
        W16all = T_pool.tile([128, KK * NCHUNK * 128], I16)  # wrapped gather idx
        W16v = W16all[:].rearrange("p (k c t) -> p k c t", k=KK, c=NCHUNK)
        Tsb = T_pool.tile([128, KK * 4 * 128], BF16)       # transposed quad weights
        fl9 = T_pool.tile([128, KK * 128], F32)
        fy9 = T_pool.tile([128, KK * 128], BF16)
        fx9 = T_pool.tile([128, KK * 128], BF16)
        fl9v = fl9[:].rearrange("p (k s) -> p k s", k=KK)
        fy9v = fy9[:].rearrange("p (k s) -> p k s", k=KK)
        fx9v = fx9[:].rearrange("p (k s) -> p k s", k=KK)
        Wq4 = T_pool.tile([128, KK * 4 * 128], BF16)
        Wq4v = Wq4[:].rearrange("p (k q s) -> p k q s", k=KK, q=4)

        xpad_t = T_pool.tile([64, 130 * 130], F16)
        xpv = xpad_t[:].rearrange("p (r c) -> p r c", c=130)

        tab = ins["tab"]       # host-built table [NRECP, 256] bf16 (ExternalInput)
        rdram = ins["rdram"]   # internal staging for quad weights

        # ---------------- pools (all open for the whole pipeline) --------
        xr_pool = ctx.enter_context(tc.tile_pool(name="xr", bufs=4))
        offs_pool = ctx.enter_context(tc.tile_pool(name="offs", bufs=2))
        ixp2 = ctx.enter_context(tc.tile_pool(name="ixq", bufs=2))
        g_pool = ctx.enter_context(tc.tile_pool(name="g", bufs=7))
        wsb_pool = ctx.enter_context(tc.tile_pool(name="wsb", bufs=2))
        r_pool = ctx.enter_context(tc.tile_pool(name="rsl", bufs=2))
        osb_pool = ctx.enter_context(tc.tile_pool(name="osb", bufs=2))
        tch_pool = ctx.enter_context(tc.tile_pool(name="tch", bufs=2))
        # PSUM bytes/partition: ops 8K + shared ps pool 2x4K = 16K exactly.
        # Every small PSUM use (conv acc, transposes, wrap, weight expansion)
        # carves a slice out of a [128, 1024] f32 "ps" tile.
        ops_pool = ctx.enter_context(tc.tile_pool(name="ops", bufs=1, space="PSUM"))
        ps_pool = ctx.enter_context(tc.tile_pool(name="ps", bufs=2, space="PSUM"))

        def ps_tile():
            return ps_pool.tile([128, 1024], F32, tag="ps", name="pst")

        # ---------------- xpad load ----------------
        nc.vector.memset(xpv[:, 0, :], 0.0)
        nc.vector.memset(xpv[:, 129, :], 0.0)
        nc.vector.memset(xpv[:, 1:129, 0], 0.0)
        nc.vector.memset(xpv[:, 1:129, 129], 0.0)
        xin = ins["x"].rearrange("p (h w) -> p h w", w=128)
        for qh in range(8):
            nc.sync.dma_start(
                xpv[:, 1 + qh * 16:1 + (qh + 1) * 16, 1:129],
                xin[:, qh * 16:(qh + 1) * 16, :])

        r_dmas_q = [[] for _ in range(NQ)]

        # ---------------- prologue parts (quarter granularity) -----------
        # tap pairs (0,1) (3,4) (6,7) differ by +1 in kx, so one +1-shifted
        # second rhs half serves all three 128-deep matmuls
        PAIRS = [(0, True), (3, True), (6, True),
                 (2, False), (5, False), (8, False)]

        def conv_part(qq, half):
            for hch in range(half * 4, half * 4 + 4):
                cch = qq * 8 + hch
                y0 = cch * 4
                xr = xr_pool.tile([128, 6 * 130], F16, tag="xr")
                nc.vector.tensor_copy(xr[:64, :],
                                      xpad_t[:, y0 * 130:(y0 + 6) * 130])
                ln = min(780, 130 * 130 - (y0 * 130 + 1))
                nc.vector.tensor_copy(
                    bass.AP(xr.tensor, xr[:].offset + 64 * 780,
                            [[780, 64], [1, ln]]),
                    xpad_t[:, y0 * 130 + 1:y0 * 130 + 1 + ln])
                ps_t = ps_tile()
                ps = ps_t[:18, :512]
                for i, (ka, paired) in enumerate(PAIRS):
                    ky, kx = ka // 3, ka % 3
                    nparts = 128 if paired else 64
                    src = bass.AP(
                        xr.tensor, xr[:].offset + ky * 130 + kx,
                        [[6 * 130, nparts], [130, 4], [1, 128]])
                    nc.tensor.matmul(
                        ps, woff_v[:nparts, i, :], src,
                        start=(i == 0), stop=(i == len(PAIRS) - 1))
                offs16 = offs_pool.tile([18, 512], F32, tag="offs")
                nc.scalar.copy(offs16[:], ps)
                # transpose this hch's 4 s-cols straight into T3
                tp_t = ps_tile()
                tp = tp_t[:, :4 * 18]
                for j4 in range(4):
                    nc.tensor.transpose(
                        tp[:, j4 * 18:(j4 + 1) * 18],
                        offs16[:, j4 * 128:(j4 + 1) * 128], ident[:18, :18])
                s = qq * 32 + hch * 4
                nc.scalar.copy(
                    T3[:, s:s + 4, :],
                    tp.rearrange("p (a c) -> p a c", a=4))

        def idx_part(qq):
            sl = slice(qq * 32, qq * 32 + 32)
            rne = 0.5 if CAST_RNE else 0.0
            for k in range(KK):
                ky, kx = k // 3, k % 3
                dy = T3[:, sl, 2 * k]
                dx = T3[:, sl, 2 * k + 1]
                ysp8 = ixp2.tile([128, 32], F32, tag="ysp8")
                nc.vector.tensor_tensor(ysp8[:], dy, siota[:, sl], ALU.add)
                nc.vector.tensor_scalar(ysp8[:], ysp8[:],
                                        float(ky + 7) - rne, None, ALU.add)
                yint = ixp2.tile([128, 32], I32, tag="yint")
                nc.vector.tensor_copy(yint[:], ysp8[:])
                y0f = ixp2.tile([128, 32], F32, tag="y0f")
                nc.vector.tensor_copy(y0f[:], yint[:])
                nc.vector.scalar_tensor_tensor(
                    fy9v[:, k, sl], ysp8[:], 0.5 if CAST_RNE else 0.0,
                    y0f[:], ALU.add, ALU.subtract)
                yi = ixp2.tile([128, 32], F32, tag="yi")
                nc.vector.tensor_scalar(yi[:], y0f[:], 6.0, 137.0,
                                        ALU.max, ALU.min)

                xsp8 = ixp2.tile([128, 32], F32, tag="xsp8")
                nc.vector.tensor_scalar(xsp8[:], dx, piota[:],
                                        float(kx + 7) - rne,
                                        ALU.add, ALU.add)
                xint = ixp2.tile([128, 32], I32, tag="xint")
                nc.vector.tensor_copy(xint[:], xsp8[:])
                x0f = ixp2.tile([128, 32], F32, tag="x0f")
                nc.vector.tensor_copy(x0f[:], xint[:])
                nc.vector.scalar_tensor_tensor(
                    fx9v[:, k, sl], xsp8[:], 0.5 if CAST_RNE else 0.0,
                    x0f[:], ALU.add, ALU.subtract)
                xi = ixp2.tile([128, 32], F32, tag="xi")
                nc.vector.tensor_scalar(xi[:], x0f[:], 6.0, 138.0,
                                        ALU.max, ALU.min)
                # record idx = (xi-6)*132 + (yi-6)
                nc.vector.tensor_scalar(fl9v[:, k, sl], xi[:], 132.0,
                                        float(6 * 132 + 6),
                                        ALU.mult, ALU.subtract)
                nc.vector.tensor_tensor(fl9v[:, k, sl], fl9v[:, k, sl],
                                        yi[:], ALU.add)

        def wrapquad_part(qq):
            sl = slice(qq * 32, qq * 32 + 32)
            # wrap record indices: they gate the gather stream
            for k in range(KK):
                wps_t = ps_tile()
                wpsk = wps_t[:, :256]
                wv = wpsk.rearrange("p (a b) -> p a b", a=8)
                for pp in range(8):
                    nc.tensor.matmul(wv[:, pp, :], msel_v[:, pp, :],
                                     fl9v[:, k, sl], start=True, stop=True)
                # W16 col (within k): chunk*128 + s*8 + pp, chunks qq*2..qq*2+1
                dstw = bass.AP(W16all.tensor,
                               W16all[:].offset + k * (NCHUNK * 128)
                               + qq * 2 * 128,
                               [[KK * NCHUNK * 128, 128], [128, 2], [8, SS],
                                [1, 8]])
                srcw = wpsk.rearrange("p (a c u) -> p c u a", a=8, c=2)
                if k % 2 == 0:
                    nc.vector.tensor_copy(dstw, srcw)
                else:
                    nc.scalar.copy(dstw, srcw)

            for k in range(KK):
                fy = fy9v[:, k, sl]
                fx = fx9v[:, k, sl]
                # quadrant weights (record order: q0=y0x0 q1=y0x1 q2=y1x0 q3=y1x1)
                wy0 = ixp2.tile([128, 32], F32, tag="wy0")
                nc.vector.tensor_scalar(wy0[:], fy, -1.0, 1.0, ALU.mult, ALU.add)
                wx0 = ixp2.tile([128, 32], F32, tag="wx0")
                nc.vector.tensor_scalar(wx0[:], fx, -1.0, 1.0, ALU.mult, ALU.add)
                nc.vector.tensor_tensor(Wq4v[:, k, 0, sl], wy0[:], wx0[:], ALU.mult)
                nc.vector.tensor_tensor(Wq4v[:, k, 1, sl], wy0[:], fx, ALU.mult)
                nc.vector.tensor_tensor(Wq4v[:, k, 2, sl], fy, wx0[:], ALU.mult)
                nc.vector.tensor_tensor(Wq4v[:, k, 3, sl], fy, fx, ALU.mult)

                # transpose quad weights -> Tsb[s-rows, (k, q, p)]
                tq_t = ps_tile()
                tpq = tq_t[:32, :256].bitcast(BF16)
                for q in range(4):
                    nc.tensor.transpose(tpq[:, q * 128:(q + 1) * 128],
                                        Wq4v[:, k, q, sl], identb[:])
                nc.scalar.copy(Tsb[qq * 32:(qq + 1) * 32,
                                   k * 512:(k + 1) * 512], tpq)

            # FOUR batched rdram writes per quarter, one per quadrant (keeps
            # the SP sequencer's per-DMA issue cost off the R36/out path)
            for q in range(4):
                d = nc.sync.dma_start(
                    bass.AP(rdram.tensor, q * KK * 16384 + qq * 32 * 128,
                            [[128, 32], [16384, KK], [1, 128]]),
                    bass.AP(Tsb.tensor,
                            Tsb[:].offset + qq * 32 * (KK * 4 * 128) + q * 128,
                            [[KK * 4 * 128, 32], [512, KK], [1, 128]]))
                r_dmas_q[qq].append(d)

        def prologue_part(qq, i):
            if i == 0:
                conv_part(qq, 0)
            elif i == 1:
                conv_part(qq, 1)
            elif i == 2:
                idx_part(qq)
            else:
                wrapquad_part(qq)

        # ---------------- main chunk: gather / blend / conv --------------
        tab_src = bass.AP(tab.tensor, 0, [[256, NRECP], [1, 256]])
        ni_reg = nc.gpsimd.to_reg(NI)

        def main_chunk(ch):
            qq = ch // 2
            n = NI
            OPS = ops_pool.tile([64, n], F32, tag="ops")
            # this chunk's weight rows in one DMA: [36=(q,k), (s, p)]
            R36 = r_pool.tile([36, n], BF16, tag="rsl")
            dr = nc.sync.dma_start(
                R36[:].rearrange("p (s c) -> p s c", c=128),
                bass.AP(rdram.tensor, ch * NI,
                        [[16384, 4 * KK], [128, n // 128], [1, 128]]))
            for d in r_dmas_q[qq]:
                add_dep_helper(dr.ins, d.ins, reason="R after rdram")
            for k in range(KK):
                G = g_pool.tile([128, 2 * n], BF16, tag="g")
                G3 = G[:].rearrange("p (m i) -> p m i", m=2)
                gi = nc.gpsimd.dma_gather(
                    G3, tab_src, W16v[:, k, ch, :],
                    n, ni_reg, 256,
                    transpose=True, single_packet=False)
                touch = tch_pool.tile([128, 2], BF16, tag="tch")
                touch_i = nc.vector.tensor_copy(touch[:], G3[:, 0, 0:2])

                # expand W[p, i] = R36[(p%4)*9+k, i] on PE (f32 PSUM in two
                # 1024-col rounds), cast to bf16 SBUF on Act, blend on DVE
                Wsb = wsb_pool.tile([128, n], BF16, tag="wsb")
                for b in range(2):
                    WPS = ps_tile()
                    for g4 in range(2):
                        nc.tensor.matmul(
                            WPS[:, g4 * 512:(g4 + 1) * 512],
                            sel36_v[:, k, :],
                            R36[:, b * 1024 + g4 * 512:
                                b * 1024 + (g4 + 1) * 512],
                            start=True, stop=True)
                    nc.scalar.copy(Wsb[:, b * 1024:(b + 1) * 1024], WPS[:])
                    for m in range(2):
                        sl = slice(b * 1024, (b + 1) * 1024)
                        bl = nc.vector.tensor_tensor(
                            G3[:, m, sl], G3[:, m, sl],
                            Wsb[:, b * 1024:(b + 1) * 1024], ALU.mult)
                        if b == 0 and m == 0:
                            add_dep_helper(bl.ins, touch_i.ins, sync=False,
                                           reason="order blend after gather-touch")
                for m in range(2):
                    for g4 in range(n // 512):
                        nc.tensor.matmul(
                            OPS[:, g4 * 512:(g4 + 1) * 512],
                            wdrep_v[:, k * 2 + m, :],
                            G3[:, m, g4 * 512:(g4 + 1) * 512],
                            start=(k == 0 and m == 0),
                            stop=(k == KK - 1 and m == 1))

            osb = osb_pool.tile([64, n], F32, tag="osb")
            nc.scalar.copy(osb[:], OPS[:])
            nc.sync.dma_start(
                outs["out"][:, ch * NI:(ch + 1) * NI], osb[:])

        # ---------------- pipelined schedule ----------------
        # Two-quarter software pipeline: conv of quarter q+2 and idx/wrap of
        # quarter q+1 are emitted inside quarter q's window, so the index
        # math never waits on its conv in-quarter (T3 finished a quarter
        # earlier) and W16/rdram are ready when the next quarter's gathers
        # reach the head of the Pool queue.
        def C(qq):
            prologue_part(qq, 0)
            prologue_part(qq, 1)

        def I(qq):
            prologue_part(qq, 2)
            prologue_part(qq, 3)

        C(0); I(0); C(1)
        main_chunk(0); C(2); I(1); main_chunk(1)
        main_chunk(2); C(3); I(2); main_chunk(3)
        main_chunk(4); I(3); main_chunk(5)
        main_chunk(6); main_chunk(7)


# ======================= host-side wrapper =======================

def build_table(xb):
    """Host-built gather table: pure relayout of one sample's x.

    Record (c, r) at row c*132 + r holds, channel-interleaved elem=ch*4+q:
    q0=x[r-2,c-2] q1=x[r-2,c-1] q2=x[r-1,c-2] q3=x[r-1,c-1], zeros outside.
    """
    xb16 = xb.astype(ml_dtypes.bfloat16)
    pad = np.zeros((C, 133, 134), dtype=ml_dtypes.bfloat16)
    pad[:, 2:130, 2:130] = xb16
    T = np.empty((LCc, LRr, C, 4), dtype=ml_dtypes.bfloat16)
    for q in range(4):
        T[:, :, :, q] = pad[:, q // 2:q // 2 + LRr,
                            q % 2:q % 2 + LCc].transpose(2, 1, 0)
    tabn = np.zeros((NRECP, 256), dtype=ml_dtypes.bfloat16)
    tabn[:NREC] = T.reshape(NREC, 256)
    return tabn


def prep_core_inputs(xb, w_offset, w_deform):
    """Per-core device inputs from one sample."""
    C_ = xb.shape[0]
    x = np.ascontiguousarray(
        xb.reshape(C_, -1).astype(np.float16))

    woff = w_offset.reshape(18, C_, KK).transpose(2, 1, 0)   # [k, c, 18]
    wpair = np.zeros((6, 128, 18), np.float32)
    for i, (ka, kb) in enumerate([(0, 1), (3, 4), (6, 7),
                                  (2, None), (5, None), (8, None)]):
        wpair[i, 0:64, :] = woff[ka]
        if kb is not None:
            wpair[i, 64:128, :] = woff[kb]
    wpair = np.ascontiguousarray(wpair)

    wd = w_deform.reshape(64, C_, KK)          # [o, c, k]
    wdt = wd.transpose(2, 1, 0)                # [k, c, o]
    arr = np.zeros((KK, 2, 32, 4, 64), np.float32)
    arr[:] = wdt.reshape(KK, 2, 32, 1, 64)
    wdrep = np.ascontiguousarray(
        arr.reshape(KK * 2, 128, 64)).astype(ml_dtypes.bfloat16)
    return {"x": x, "wpair": wpair, "wdrep": wdrep, "tab": build_table(xb)}


_NC_CACHE = {}


def _build_nc():
    if "nc" in _NC_CACHE:
        return _NC_CACHE["nc"]
    nc = bacc.Bacc("TRN2", target_bir_lowering=False, debug=False,
                   num_devices=NCORES)
    ins = {
        "x": nc.dram_tensor("x", [C, HW], F16, kind="ExternalInput").ap(),
        "wpair": nc.dram_tensor("wpair", [6, 128, 18], F32, kind="ExternalInput").ap(),
        "wdrep": nc.dram_tensor("wdrep", [KK * 2, 128, 64], BF16, kind="ExternalInput").ap(),
        "tab": nc.dram_tensor("tab", [NRECP, 256], BF16, kind="ExternalInput").ap(),
        "rdram": nc.dram_tensor("rdram", [4 * KK, 16384], BF16, kind="Internal").ap(),
    }
    outs = {"out": nc.dram_tensor("out", [C, HW], F32, kind="ExternalOutput").ap()}
    with tile.TileContext(nc, trace_sim=False) as tc:
        build_kernel(tc, outs, ins)
    nc.compile()
    _NC_CACHE["nc"] = nc
    return nc


def kernel(x, w_offset, w_deform):
    x = np.asarray(x, dtype=np.float32)
    w_offset = np.asarray(w_offset, dtype=np.float32)
    w_deform = np.asarray(w_deform, dtype=np.float32)
    nc = _build_nc()
    in_maps = [prep_core_inputs(x[b], w_offset, w_deform) for b in range(B)]
    res = bass_utils.run_bass_kernel_spmd(nc, in_maps, core_ids=list(range(NCORES)))
    out = np.stack([res.results[b]["out"].reshape(C, H, W) for b in range(B)])
    return out.astype(np.float32)
